# revision 6
# baseline (speedup 1.0000x reference)
"""CRF-RNN layer (nn_CrfRnnLayer) as a Bass/Tile SPMD kernel on 8 TRN2 NeuronCores.

Algorithm (matches reference.py):
  N = 112*112 pixels, C = 21 classes, 5 mean-field iterations:
    sm = softmax(Q, axis=classes)
    spatial_out  = (sm @ Ks) / ns      Ks[i,j] = exp(-||p_i-p_j||^2 / (2*3^2))
    bilateral_out= (sm @ Kb) / nb      Kb from (pos/160, rgb/3) features
    Q = u - comp @ (sk @ spatial_out + bk @ bilateral_out)

Sharding: pixel columns sharded 8 ways (each core owns 14 image rows = 1568
pixels). Each core computes its column slice of both filtered outputs from the
fully replicated softmax; a per-iteration AllGather (131KB/rank) replicates Q.

Structure per core (instruction-count optimized):
  - Bilateral slice E_b [N, 1568] precomputed once on-device in bf16 (K=7
    matmul of augmented features -> -0.5*d2 into a 4-bank PSUM tile, one ACT
    exp over all 4 banks) and streamed back each iteration in 7-block batches.
  - Main bilateral matmul: 98 contraction blocks x 4 col-tiles into a single
    4-bank [33, 2048] PSUM accumulator; softmax lhsT carries a ones column at
    partition 32 whose PSUM row is the nb normalizer (read once, iteration 0).
  - Spatial kernel never materialized: Ks = Gy (x) Gx Kronecker product. One
    big DVE multiply (per half) scales the softmax image by the per-core
    y-Gaussian weights; 112 PE matmuls against the shared [112,112] x-Gaussian
    do the rest. Normalizers ns are exact separable sums computed on host.
  - Layout changes (pixel-major <-> block-major <-> class-major) via single
    strided DMAs through DRAM instead of per-tile PE transposes.
"""

import numpy as np

import concourse.mybir as mybir
import concourse.tile as tile
from concourse import bacc
from concourse.bass_utils import run_bass_kernel_spmd

H = 112
W = 112
C = 21
N = H * W
NCORES = 8
YPC = H // NCORES            # 14 image rows per core
COLS = N // NCORES           # 1568 pixels per core
NB = 98                      # contraction blocks of 128 pixels
BB = 7                       # blocks per DMA batch
NBATCH = NB // BB            # 14
CTS = [(0, 512), (512, 512), (1024, 512), (1536, 32)]
NITER = 5
THETA_ALPHA = 160.0
THETA_BETA = 3.0
THETA_GAMMA = 3.0
CP = 33                      # padded lhsT width; col 32 is the ones column

F32 = mybir.dt.float32
BF16 = mybir.dt.bfloat16
EXPF = mybir.ActivationFunctionType.Exp

_CACHE = {}


def _build_program(reps=1):
    nc = bacc.Bacc("TRN2", target_bir_lowering=False, debug=False, num_devices=NCORES)

    ubT = nc.dram_tensor("ubT", [7, N], F32, kind="ExternalInput")
    vbT_sl = nc.dram_tensor("vbT_sl", [7, COLS], F32, kind="ExternalInput")
    g2d = nc.dram_tensor("g2d", [H, W], F32, kind="ExternalInput")
    gmat = nc.dram_tensor("gmat", [W, H * YPC], F32, kind="ExternalInput")
    invns = nc.dram_tensor("invns", [W, YPC * C], F32, kind="ExternalInput")
    u_sl = nc.dram_tensor("u_sl", [C, COLS], F32, kind="ExternalInput")
    qt0 = nc.dram_tensor("qt0", [W, H * C], F32, kind="ExternalInput")
    asT = nc.dram_tensor("asT", [C, C], F32, kind="ExternalInput")
    abT = nc.dram_tensor("abT", [C, C], F32, kind="ExternalInput")
    qt_out = nc.dram_tensor("qt_out", [C, COLS], F32, kind="ExternalOutput")

    with tile.TileContext(nc) as tc:
        with (
            tc.tile_pool(name="const", bufs=1) as cpool,
            tc.tile_pool(name="smx", bufs=1) as smpool,
            tc.tile_pool(name="stream", bufs=2) as stpool,
            tc.tile_pool(name="outp", bufs=1) as opool,
            tc.tile_pool(name="psum", bufs=1, space="PSUM") as pspool,
            tc.tile_pool(name="dram", bufs=1, space="DRAM") as dpool,
        ):
          for _rep in range(reps):
            # ---------------- constants ----------------
            vbT_sb = cpool.tile([7, COLS], F32, tag="vbT", name=f"vbT_{_rep}")
            nc.sync.dma_start(vbT_sb[:], vbT_sl[:])
            u_sb = cpool.tile([C, COLS], F32, tag="usb", name=f"usb_{_rep}")
            nc.sync.dma_start(u_sb[:], u_sl[:])
            asT_sb = cpool.tile([C, C], F32, tag="asT", name=f"asT_{_rep}")
            nc.sync.dma_start(asT_sb[:], asT[:])
            abT_sb = cpool.tile([C, C], F32, tag="abT", name=f"abT_{_rep}")
            nc.sync.dma_start(abT_sb[:], abT[:])
            invns_sb = cpool.tile([W, YPC * C], F32, tag="invns", name=f"invns_{_rep}")
            nc.sync.dma_start(invns_sb[:], invns[:])

            g2d_f = cpool.tile([H, W], F32, tag="g2df", name=f"g2df_{_rep}")
            nc.sync.dma_start(g2d_f[:], g2d[:])
            g2d_bf = cpool.tile([H, W], BF16, tag="g2db", name=f"g2db_{_rep}")
            nc.vector.tensor_copy(g2d_bf[:], g2d_f[:])

            gmat_f = cpool.tile([W, H * YPC], F32, tag="gmf", name=f"gmf_{_rep}")
            nc.sync.dma_start(gmat_f[:], gmat[:])
            gmat_bf = cpool.tile([W, H, YPC], BF16, tag="gmb", name=f"gmb_{_rep}")
            nc.vector.tensor_copy(
                gmat_bf[:], gmat_f[:].rearrange("p (y k) -> p y k", y=H)
            )

            ones1 = cpool.tile([1, C], F32, tag="ones1", name=f"ones1_{_rep}")
            nc.gpsimd.memset(ones1[:], 1.0)
            invnb_bc = cpool.tile([C, COLS], F32, tag="invnb", name=f"invnb_{_rep}")

            # DRAM scratch
            e_b = dpool.tile([NB, 128, COLS], BF16, tag="eb", name=f"eb_{_rep}")
            sm_d = dpool.tile([N, C], BF16, tag="smd", name=f"smd_{_rep}")
            spb = dpool.tile([YPC * C, W], F32, tag="spb", name=f"spb_{_rep}")

            # ---------------- precompute E_b (98 blocks, batches of 7) ------
            for bt in range(NBATCH):
                ub7 = stpool.tile([7, BB * 128], F32, tag="ub7", name=f"ub7_{_rep}_{bt}")
                nc.sync.dma_start(
                    ub7[:], ubT[:, bt * BB * 128 : (bt + 1) * BB * 128]
                )
                e7 = stpool.tile(
                    [128, BB, COLS], BF16, tag="e7", name=f"e7p_{_rep}_{bt}"
                )
                for b in range(BB):
                    tagp = "blk" if b % 2 == 0 else "spq"
                    d2_ps = pspool.tile(
                        [128, 2048], F32, tag=tagp, name=f"d2_{_rep}_{bt}_{b}"
                    )
                    for ci, (c0, cw) in enumerate(CTS):
                        nc.tensor.matmul(
                            d2_ps[:, ci * 512 : ci * 512 + cw],
                            ub7[:, b * 128 : (b + 1) * 128],
                            vbT_sb[:, c0 : c0 + cw],
                            start=True,
                            stop=True,
                        )
                    nc.scalar.activation(e7[:, b, :], d2_ps[:, 0:COLS], EXPF)
                nc.sync.dma_start(
                    e_b[bt * BB : (bt + 1) * BB].rearrange("b r f -> r b f"), e7[:]
                )

            # ---------------- iterations ----------------
            qt_full = None
            for it in range(NITER):
                # ---- Q in [x, (y c)] layout
                q3f = smpool.tile([W, H * C], F32, tag="q3f", name=f"q3f_{_rep}_{it}")
                if it == 0:
                    nc.sync.dma_start(q3f[:], qt0[:])
                else:
                    nc.sync.dma_start(
                        q3f[:], qt_full[:].rearrange("r x -> x r")
                    )

                # ---- softmax over classes (no max subtraction; |Q| small)
                eq3b = smpool.tile([W, H * C], BF16, tag="eq3b", name=f"eq_{_rep}_{it}")
                nc.scalar.activation(eq3b[:], q3f[:], EXPF)
                sums = smpool.tile([W, H], F32, tag="sums", name=f"sums_{_rep}_{it}")
                nc.vector.reduce_sum(
                    sums[:],
                    eq3b[:].rearrange("p (y c) -> p y c", y=H),
                    axis=mybir.AxisListType.X,
                )
                rsum = smpool.tile([W, H], F32, tag="rsum", name=f"rsum_{_rep}_{it}")
                nc.vector.reciprocal(rsum[:], sums[:])
                rsumb = smpool.tile([W, H], BF16, tag="rsumb", name=f"rsumb_{_rep}_{it}")
                nc.vector.tensor_copy(rsumb[:], rsum[:])
                smT3 = smpool.tile([W, H, C], BF16, tag="smT3", name=f"smT3_{_rep}_{it}")
                nc.vector.tensor_mul(
                    smT3[:],
                    eq3b[:].rearrange("p (y c) -> p y c", y=H),
                    rsumb[:].broadcast_to([W, H, C]),
                )

                # ---- block-major softmax copy for the bilateral lhsT
                nc.sync.dma_start(
                    sm_d[:].rearrange("(y x) c -> x y c", x=W), smT3[:]
                )
                smB = smpool.tile([128, NB, CP], BF16, tag="smB", name=f"smB_{_rep}_{it}")
                nc.gpsimd.memset(smB[:, :, C:CP], 1.0)
                nc.sync.dma_start(
                    smB[:, :, 0:C], sm_d[:].rearrange("(b r) c -> r b c", r=128)
                )

                # ---- spatial filtering (Kronecker, never materialized)
                sp_ps = pspool.tile([W, YPC * C], F32, tag="spq", name=f"sp_{_rep}_{it}")
                sp_ps3 = sp_ps[:].rearrange("p (k c) -> p k c", k=YPC)
                for half in range(2):
                    y0h = half * (H // 2)
                    srhs = stpool.tile(
                        [W, H // 2, YPC, C], BF16, tag="srhs", bufs=1,
                        name=f"srhs_{_rep}_{it}_{half}",
                    )
                    nc.vector.tensor_mul(
                        srhs[:],
                        smT3[:, y0h : y0h + H // 2, :]
                        .rearrange("p y (o c) -> p y o c", o=1)
                        .broadcast_to([W, H // 2, YPC, C]),
                        gmat_bf[:, y0h : y0h + H // 2, :]
                        .rearrange("p y (k o) -> p y k o", o=1)
                        .broadcast_to([W, H // 2, YPC, C]),
                    )
                    for yl in range(H // 2):
                        y = y0h + yl
                        nc.tensor.matmul(
                            sp_ps3[:],
                            g2d_bf[:],
                            srhs[:, yl, :, :],
                            start=(y == 0),
                            stop=(y == H - 1),
                        )

                # ---- bilateral: stream E_b and accumulate [CP, 2048] PSUM
                bl_ps = pspool.tile([CP, 2048], F32, tag="blk", name=f"bl_{_rep}_{it}")
                for bt in range(NBATCH):
                    e7s = stpool.tile(
                        [128, BB, COLS], BF16, tag="e7", name=f"e7s_{_rep}_{it}_{bt}"
                    )
                    nc.sync.dma_start(
                        e7s[:], e_b[bt * BB : (bt + 1) * BB].rearrange("b r f -> r b f")
                    )
                    for b in range(BB):
                        jb = bt * BB + b
                        for ci, (c0, cw) in enumerate(CTS):
                            nc.tensor.matmul(
                                bl_ps[:, ci * 512 : ci * 512 + cw],
                                smB[:, jb, :],
                                e7s[:, b, c0 : c0 + cw],
                                start=(jb == 0),
                                stop=(jb == NB - 1),
                            )

                # ---- iteration 0: build 1/nb broadcast across class partitions
                if it == 0:
                    nbrow = opool.tile([1, COLS], F32, tag="nbrow", name=f"nbrow_{_rep}")
                    nc.vector.tensor_copy(nbrow[:], bl_ps[32:33, 0:COLS])
                    rnb = opool.tile([1, COLS], F32, tag="rnb", name=f"rnb_{_rep}")
                    nc.vector.reciprocal(rnb[:], nbrow[:])
                    bc_ps = pspool.tile([C, 2048], F32, tag="spq", name=f"bc_{_rep}")
                    for ci, (c0, cw) in enumerate(CTS):
                        nc.tensor.matmul(
                            bc_ps[:, ci * 512 : ci * 512 + cw],
                            ones1[:],
                            rnb[0:1, c0 : c0 + cw],
                            start=True,
                            stop=True,
                        )
                    nc.vector.tensor_copy(invnb_bc[:], bc_ps[:, 0:COLS])

                # ---- normalize spatial, repartition [x,(k,c)] -> [c, cols]
                sp_n = opool.tile([W, YPC * C], F32, tag="spn", name=f"spn_{_rep}_{it}")
                nc.vector.tensor_mul(sp_n[:], sp_ps[:], invns_sb[:])
                # 2D-transpose write: spb[(k c), x] <- sp_n[x, (k c)]
                nc.sync.dma_start(spb[:].rearrange("r x -> x r"), sp_n[:])
                # 3D contiguous-inner read: sp_out[c, (k x)] <- spb[(k c), x]
                sp_out = opool.tile([C, COLS], F32, tag="spo", name=f"spo_{_rep}_{it}")
                nc.sync.dma_start(
                    sp_out[:].rearrange("c (k x) -> c k x", k=YPC),
                    spb[:].rearrange("(k c) x -> c k x", k=YPC),
                )

                # ---- normalize bilateral
                bl_out = opool.tile([C, COLS], F32, tag="blo", name=f"blo_{_rep}_{it}")
                nc.vector.tensor_mul(bl_out[:], bl_ps[0:C, 0:COLS], invnb_bc[:])

                # ---- Q = u + A_s @ sp_out + A_b @ bl_out
                q_ps = pspool.tile([C, 2048], F32, tag="spq", name=f"qps_{_rep}_{it}")
                for ci, (c0, cw) in enumerate(CTS):
                    nc.tensor.matmul(
                        q_ps[:, ci * 512 : ci * 512 + cw],
                        asT_sb[:],
                        sp_out[:, c0 : c0 + cw],
                        start=True,
                        stop=False,
                    )
                for ci, (c0, cw) in enumerate(CTS):
                    nc.tensor.matmul(
                        q_ps[:, ci * 512 : ci * 512 + cw],
                        abT_sb[:],
                        bl_out[:, c0 : c0 + cw],
                        start=False,
                        stop=True,
                    )
                q_sb = opool.tile([C, COLS], F32, tag="qsb", name=f"qsb_{_rep}_{it}")
                nc.vector.tensor_add(q_sb[:], q_ps[:, 0:COLS], u_sb[:])

                # ---- publish Q: AllGather (iters 0-3) or final output
                if it < NITER - 1:
                    qt_sl = dpool.tile(
                        [YPC * C, W], F32, tag="qtsl", bufs=2, name=f"qtsl_{_rep}_{it}"
                    )
                    nc.sync.dma_start(
                        qt_sl[:].rearrange("(k c) x -> c k x", k=YPC),
                        q_sb[:].rearrange("c (k x) -> c k x", k=YPC),
                    )
                    qt_full = dpool.tile(
                        [H * C, W], F32, tag="qtfull", bufs=2,
                        addr_space="Shared", name=f"qtfull_{_rep}_{it}",
                    )
                    nc.gpsimd.collective_compute(
                        "AllGather",
                        mybir.AluOpType.bypass,
                        replica_groups=[list(range(NCORES))],
                        ins=[qt_sl[:]],
                        outs=[qt_full[:]],
                    )
                else:
                    nc.sync.dma_start(qt_out[:], q_sb[:])

    nc.compile()
    return nc


def _host_inputs(unaries, rgb, spatial_kernel, bilateral_kernel, compatibility_matrix):
    u = np.transpose(np.asarray(unaries, dtype=np.float32)[0], (2, 0, 1)).reshape(C, N)
    rgbf = np.asarray(rgb, dtype=np.float32)[0].reshape(N, 3)

    yy, xx = np.meshgrid(
        np.arange(H, dtype=np.float64), np.arange(W, dtype=np.float64), indexing="ij"
    )
    pos = np.stack([xx.ravel(), yy.ravel()], axis=1)  # [N, 2] (x, y)

    fb = np.concatenate(
        [pos / THETA_ALPHA, rgbf.astype(np.float64) / THETA_BETA], axis=1
    )
    fb -= fb.mean(axis=0, keepdims=True)  # centering: reduces fp32 cancellation
    fb32 = fb.astype(np.float32)
    sq = (fb32.astype(np.float64) ** 2).sum(axis=1)
    mhalf_sq = (-0.5 * sq).astype(np.float32)

    ubT = np.empty((7, N), np.float32)
    ubT[0:5] = fb32.T
    ubT[5] = mhalf_sq
    ubT[6] = 1.0
    vbT = np.empty((7, N), np.float32)
    vbT[0:5] = fb32.T
    vbT[5] = 1.0
    vbT[6] = mhalf_sq

    d = np.arange(-(H - 1), H, dtype=np.float64)
    g1tab = np.exp(-(d * d) / (2.0 * THETA_GAMMA**2))

    def g1(dd):
        return g1tab[np.asarray(dd) + (H - 1)]

    gx = g1(np.arange(W)[:, None] - np.arange(W)[None, :])  # [x, x']
    g2d_np = gx.astype(np.float32)
    s1 = np.array([g1(np.arange(H) - t).sum() for t in range(H)])  # exact ns factors

    comp = np.asarray(compatibility_matrix, dtype=np.float64)
    A_s = -(comp @ np.asarray(spatial_kernel, dtype=np.float64))
    A_b = -(comp @ np.asarray(bilateral_kernel, dtype=np.float64))
    asT_np = np.ascontiguousarray(A_s.T).astype(np.float32)
    abT_np = np.ascontiguousarray(A_b.T).astype(np.float32)

    qt0_np = np.ascontiguousarray(
        u.reshape(C, H, W).transpose(2, 1, 0).reshape(W, H * C)
    )

    in_maps = []
    for c in range(NCORES):
        sl = slice(c * COLS, (c + 1) * COLS)
        dy = np.arange(H)[:, None] - (YPC * c + np.arange(YPC))[None, :]  # [y, k]
        gm = g1(dy).astype(np.float32)  # [112, 14]
        gmat_np = np.ascontiguousarray(
            np.broadcast_to(gm[None], (W, H, YPC))
        ).reshape(W, H * YPC)
        v = 1.0 / (s1[YPC * c + np.arange(YPC)][None, :] * s1[np.arange(W)][:, None])
        invns_np = np.ascontiguousarray(
            np.repeat(v[:, :, None], C, axis=2).astype(np.float32)
        ).reshape(W, YPC * C)
        in_maps.append(
            dict(
                ubT=ubT,
                vbT_sl=np.ascontiguousarray(vbT[:, sl]),
                g2d=g2d_np,
                gmat=gmat_np,
                invns=invns_np,
                u_sl=np.ascontiguousarray(u[:, sl]),
                qt0=qt0_np,
                asT=asT_np,
                abT=abT_np,
            )
        )
    return in_maps


def run(inputs, trace=False, reps=1, **spmd_kwargs):
    in_maps = _host_inputs(**inputs)
    key = ("nc", reps)
    if key not in _CACHE:
        _CACHE[key] = _build_program(reps)
    nc = _CACHE[key]
    res = run_bass_kernel_spmd(
        nc, in_maps, core_ids=list(range(NCORES)), trace=trace, **spmd_kwargs
    )
    qs = [np.asarray(res.results[c]["qt_out"]) for c in range(NCORES)]
    Q = np.concatenate(qs, axis=1)  # [C, N]
    out = Q.reshape(C, H, W).transpose(1, 2, 0)[None].astype(np.float32)
    return out, res


def kernel(unaries, rgb, spatial_kernel, bilateral_kernel, compatibility_matrix):
    out, _ = run(
        dict(
            unaries=unaries,
            rgb=rgb,
            spatial_kernel=spatial_kernel,
            bilateral_kernel=bilateral_kernel,
            compatibility_matrix=compatibility_matrix,
        )
    )
    return out


# revision 8
# speedup vs baseline: 1.4362x; 1.4362x over previous
"""CRF-RNN layer (nn_CrfRnnLayer) as a Bass/Tile SPMD kernel on 8 TRN2 NeuronCores.

Algorithm (matches reference.py):
  N = 112*112 pixels, C = 21 classes, 5 mean-field iterations:
    sm = softmax(Q, axis=classes)
    spatial_out  = (sm @ Ks) / ns      Ks[i,j] = exp(-||p_i-p_j||^2 / (2*3^2))
    bilateral_out= (sm @ Kb) / nb      Kb from (pos/160, rgb/3) features
    Q = u - comp @ (sk @ spatial_out + bk @ bilateral_out)

Sharding: pixel columns sharded 8 ways (each core owns 14 image rows = 1568
pixels). Each core computes its column slice of both filtered outputs from the
fully replicated softmax; a per-iteration AllGather (131KB/rank) replicates Q.

Structure per core (instruction-count optimized):
  - Bilateral slice E_b [N, 1568] precomputed once on-device in bf16 (K=7
    matmul of augmented features -> -0.5*d2 into a 4-bank PSUM tile, one ACT
    exp over all 4 banks) and streamed back each iteration in 7-block batches.
  - Main bilateral matmul: 98 contraction blocks x 4 col-tiles into a single
    4-bank [33, 2048] PSUM accumulator; softmax lhsT carries a ones column at
    partition 32 whose PSUM row is the nb normalizer (read once, iteration 0).
  - Spatial kernel never materialized: Ks = Gy (x) Gx Kronecker product. One
    big DVE multiply (per half) scales the softmax image by the per-core
    y-Gaussian weights; 112 PE matmuls against the shared [112,112] x-Gaussian
    do the rest. Normalizers ns are exact separable sums computed on host.
  - Layout changes (pixel-major <-> block-major <-> class-major) via single
    strided DMAs through DRAM instead of per-tile PE transposes.
"""

import numpy as np

import concourse.mybir as mybir
import concourse.tile as tile
from concourse import bacc
from concourse.bass import _add_dep_helper
from concourse.bass_utils import run_bass_kernel_spmd

H = 112
W = 112
C = 21
N = H * W
NCORES = 8
YPC = H // NCORES            # 14 image rows per core
COLS = N // NCORES           # 1568 pixels per core
NB = 98                      # contraction blocks of 128 pixels
BB = 7                       # blocks per DMA batch
NBATCH = NB // BB            # 14
CTS = [(0, 512), (512, 512), (1024, 512), (1536, 32)]
NITER = 5
THETA_ALPHA = 160.0
THETA_BETA = 3.0
THETA_GAMMA = 3.0
CP = 33                      # padded lhsT width; col 32 is the ones column

F32 = mybir.dt.float32
BF16 = mybir.dt.bfloat16
EXPF = mybir.ActivationFunctionType.Exp

_CACHE = {}


def _build_program(reps=1):
    nc = bacc.Bacc("TRN2", target_bir_lowering=False, debug=False, num_devices=NCORES)

    # Chain every PE matmul in emission order (ordering-only deps) so the
    # scheduler keeps same-weights matmuls adjacent -> legalization dedups
    # the LDWEIGHTS instruction for consecutive same-lhsT matmuls.
    _mm_state = {"prev": None}

    def mm(*args, **kwargs):
        inst = nc.tensor.matmul(*args, **kwargs)
        if _mm_state["prev"] is not None:
            _add_dep_helper(inst.ins, _mm_state["prev"].ins, sync=False,
                            reason="pe emission order")
        _mm_state["prev"] = inst
        return inst

    ubT = nc.dram_tensor("ubT", [7, N], F32, kind="ExternalInput")
    vbT_sl = nc.dram_tensor("vbT_sl", [7, COLS], F32, kind="ExternalInput")
    g2d = nc.dram_tensor("g2d", [H, W], F32, kind="ExternalInput")
    gmat = nc.dram_tensor("gmat", [W, H * YPC], F32, kind="ExternalInput")
    invns = nc.dram_tensor("invns", [W, YPC * C], F32, kind="ExternalInput")
    u_sl = nc.dram_tensor("u_sl", [C, COLS], F32, kind="ExternalInput")
    qt0 = nc.dram_tensor("qt0", [W, H * C], F32, kind="ExternalInput")
    awT = nc.dram_tensor("awT", [54, C], F32, kind="ExternalInput")
    qt_out = nc.dram_tensor("qt_out", [C, COLS], F32, kind="ExternalOutput")

    with tile.TileContext(nc) as tc:
        with (
            tc.tile_pool(name="const", bufs=1) as cpool,
            tc.tile_pool(name="smx", bufs=1) as smpool,
            tc.tile_pool(name="stream", bufs=2) as stpool,
            tc.tile_pool(name="outp", bufs=1) as opool,
            tc.tile_pool(name="psum", bufs=1, space="PSUM") as pspool,
            tc.tile_pool(name="dram", bufs=1, space="DRAM") as dpool,
        ):
          for _rep in range(reps):
            # ---------------- constants ----------------
            vbT_sb = cpool.tile([7, COLS], F32, tag="vbT", name=f"vbT_{_rep}")
            nc.sync.dma_start(vbT_sb[:], vbT_sl[:])
            u_sb = cpool.tile([C, COLS], F32, tag="usb", name=f"usb_{_rep}")
            nc.sync.dma_start(u_sb[:], u_sl[:])
            awT_sb = cpool.tile([54, C], F32, tag="awT", name=f"awT_{_rep}")
            nc.sync.dma_start(awT_sb[:], awT[:])
            invns_sb = cpool.tile([W, YPC * C], F32, tag="invns", name=f"invns_{_rep}")
            nc.sync.dma_start(invns_sb[:], invns[:])

            g2d_f = cpool.tile([H, W], F32, tag="g2df", name=f"g2df_{_rep}")
            nc.sync.dma_start(g2d_f[:], g2d[:])

            gmat_f = cpool.tile([W, H * YPC], F32, tag="gmf", name=f"gmf_{_rep}")
            nc.sync.dma_start(gmat_f[:], gmat[:])
            gmat_3 = gmat_f[:].rearrange("p (y k) -> p y k", y=H)

            ones1 = cpool.tile([1, C], F32, tag="ones1", name=f"ones1_{_rep}")
            nc.gpsimd.memset(ones1[:], 1.0)
            invnb_bc = cpool.tile([C, COLS], F32, tag="invnb", name=f"invnb_{_rep}")

            # DRAM scratch
            e_b = dpool.tile([NB, 128, COLS], F32, tag="eb", name=f"eb_{_rep}")
            sm_d = dpool.tile([N, C], F32, tag="smd", name=f"smd_{_rep}")
            spb = dpool.tile([YPC * C, W], F32, tag="spb", name=f"spb_{_rep}")

            # ---------------- precompute E_b (98 blocks, batches of 7) ------
            for bt in range(NBATCH):
                ub7 = stpool.tile([7, BB * 128], F32, tag="ub7", name=f"ub7_{_rep}_{bt}")
                nc.sync.dma_start(
                    ub7[:], ubT[:, bt * BB * 128 : (bt + 1) * BB * 128]
                )
                e7 = stpool.tile(
                    [128, BB, COLS], F32, tag="e7", bufs=1, name=f"e7p_{_rep}_{bt}"
                )
                for b in range(BB):
                    tagp = "blk" if b % 2 == 0 else "spq"
                    d2_ps = pspool.tile(
                        [128, 2048], F32, tag=tagp, name=f"d2_{_rep}_{bt}_{b}"
                    )
                    for ci, (c0, cw) in enumerate(CTS):
                        mm(
                            d2_ps[:, ci * 512 : ci * 512 + cw],
                            ub7[:, b * 128 : (b + 1) * 128],
                            vbT_sb[:, c0 : c0 + cw],
                            start=True,
                            stop=True,
                        )
                    nc.scalar.activation(e7[:, b, :], d2_ps[:, 0:COLS], EXPF)
                nc.sync.dma_start(
                    e_b[bt * BB : (bt + 1) * BB].rearrange("b r f -> r b f"), e7[:]
                )

            # ---------------- iterations ----------------
            qt_full = None
            for it in range(NITER):
                # ---- Q in [x, (y c)] layout
                q3f = smpool.tile([W, H * C], F32, tag="q3f", name=f"q3f_{_rep}_{it}")
                if it == 0:
                    nc.sync.dma_start(q3f[:], qt0[:])
                else:
                    nc.sync.dma_start(
                        q3f[:], qt_full[:].rearrange("r x -> x r")
                    )

                # ---- softmax over classes (no max subtraction; |Q| small)
                eq3b = smpool.tile([W, H * C], F32, tag="eq3b", name=f"eq_{_rep}_{it}")
                nc.scalar.activation(eq3b[:], q3f[:], EXPF)
                sums = smpool.tile([W, H], F32, tag="sums", name=f"sums_{_rep}_{it}")
                nc.vector.reduce_sum(
                    sums[:],
                    eq3b[:].rearrange("p (y c) -> p y c", y=H),
                    axis=mybir.AxisListType.X,
                )
                rsum = smpool.tile([W, H], F32, tag="rsum", name=f"rsum_{_rep}_{it}")
                nc.vector.reciprocal(rsum[:], sums[:])
                smT3 = smpool.tile([W, H, C], F32, tag="smT3", name=f"smT3_{_rep}_{it}")
                nc.vector.tensor_mul(
                    smT3[:],
                    eq3b[:].rearrange("p (y c) -> p y c", y=H),
                    rsum[:].broadcast_to([W, H, C]),
                )

                # ---- block-major softmax copy for the bilateral lhsT
                nc.sync.dma_start(
                    sm_d[:].rearrange("(y x) c -> x y c", x=W), smT3[:]
                )
                smB = smpool.tile([128, NB, CP], F32, tag="smB", name=f"smB_{_rep}_{it}")
                nc.gpsimd.memset(smB[:, :, C:CP], 1.0)
                nc.sync.dma_start(
                    smB[:, :, 0:C], sm_d[:].rearrange("(b r) c -> r b c", r=128)
                )

                # ---- spatial filtering (Kronecker, never materialized)
                sp_ps = pspool.tile([W, YPC * C], F32, tag="spq", name=f"sp_{_rep}_{it}")
                sp_ps3 = sp_ps[:].rearrange("p (k c) -> p k c", k=YPC)
                YQ = H // 4
                for quarter in range(4):
                    y0h = quarter * YQ
                    srhs = stpool.tile(
                        [W, YQ, YPC, C], F32, tag="srhs", bufs=1,
                        name=f"srhs_{_rep}_{it}_{quarter}",
                    )
                    nc.vector.tensor_mul(
                        srhs[:],
                        smT3[:, y0h : y0h + YQ, :]
                        .rearrange("p y (o c) -> p y o c", o=1)
                        .broadcast_to([W, YQ, YPC, C]),
                        gmat_3[:, y0h : y0h + YQ, :]
                        .rearrange("p y (k o) -> p y k o", o=1)
                        .broadcast_to([W, YQ, YPC, C]),
                    )
                    for yl in range(YQ):
                        y = y0h + yl
                        mm(
                            sp_ps3[:],
                            g2d_f[:],
                            srhs[:, yl, :, :],
                            start=(y == 0),
                            stop=(y == H - 1),
                        )

                # ---- bilateral: stream E_b and accumulate [CP, 2048] PSUM
                bl_ps = pspool.tile([CP, 2048], F32, tag="blk", name=f"bl_{_rep}_{it}")
                for bt in range(NBATCH):
                    e7s = stpool.tile(
                        [128, BB, COLS], F32, tag="e7", bufs=1, name=f"e7s_{_rep}_{it}_{bt}"
                    )
                    nc.sync.dma_start(
                        e7s[:], e_b[bt * BB : (bt + 1) * BB].rearrange("b r f -> r b f")
                    )
                    for b in range(BB):
                        jb = bt * BB + b
                        for ci, (c0, cw) in enumerate(CTS):
                            mm(
                                bl_ps[:, ci * 512 : ci * 512 + cw],
                                smB[:, jb, :],
                                e7s[:, b, c0 : c0 + cw],
                                start=(jb == 0),
                                stop=(jb == NB - 1),
                            )

                # ---- iteration 0: build 1/nb broadcast across class partitions
                if it == 0:
                    nbrow = opool.tile([1, COLS], F32, tag="nbrow", name=f"nbrow_{_rep}")
                    nc.vector.tensor_copy(nbrow[:], bl_ps[32:33, 0:COLS])
                    rnb = opool.tile([1, COLS], F32, tag="rnb", name=f"rnb_{_rep}")
                    nc.vector.reciprocal(rnb[:], nbrow[:])
                    bc_ps = pspool.tile([C, 2048], F32, tag="spq", name=f"bc_{_rep}")
                    for ci, (c0, cw) in enumerate(CTS):
                        mm(
                            bc_ps[:, ci * 512 : ci * 512 + cw],
                            ones1[:],
                            rnb[0:1, c0 : c0 + cw],
                            start=True,
                            stop=True,
                        )
                    nc.vector.tensor_copy(invnb_bc[:], bc_ps[:, 0:COLS])

                # ---- stacked [54, COLS]: spatial_out rows 0:21, bilateral 32:53
                so54 = opool.tile([54, COLS], F32, tag="so54", name=f"so54_{_rep}_{it}")
                nc.gpsimd.memset(so54[:], 0.0)
                sp_n = opool.tile([W, YPC * C], F32, tag="spn", name=f"spn_{_rep}_{it}")
                nc.vector.tensor_mul(sp_n[:], sp_ps[:], invns_sb[:])
                # 2D-transpose write: spb[(k c), x] <- sp_n[x, (k c)]
                nc.sync.dma_start(spb[:].rearrange("r x -> x r"), sp_n[:])
                # 3D contiguous-inner read: so54[c, (k x)] <- spb[(k c), x]
                nc.sync.dma_start(
                    so54[0:C, :].rearrange("c (k x) -> c k x", k=YPC),
                    spb[:].rearrange("(k c) x -> c k x", k=YPC),
                )
                # normalized bilateral into rows 32:53
                nc.vector.tensor_mul(so54[32:53, :], bl_ps[0:C, 0:COLS], invnb_bc[:])

                # ---- Q = u + [A_s ; A_b] @ [sp_out ; bl_out]
                q_ps = pspool.tile([C, 2048], F32, tag="spq", name=f"qps_{_rep}_{it}")
                for ci, (c0, cw) in enumerate(CTS):
                    mm(
                        q_ps[:, ci * 512 : ci * 512 + cw],
                        awT_sb[:],
                        so54[:, c0 : c0 + cw],
                        start=True,
                        stop=True,
                    )
                q_sb = opool.tile([C, COLS], F32, tag="qsb", name=f"qsb_{_rep}_{it}")
                nc.vector.tensor_add(q_sb[:], q_ps[:, 0:COLS], u_sb[:])

                # ---- publish Q: AllGather (iters 0-3) or final output
                if it < NITER - 1:
                    qt_sl = dpool.tile(
                        [YPC * C, W], F32, tag="qtsl", bufs=2, name=f"qtsl_{_rep}_{it}"
                    )
                    nc.sync.dma_start(
                        qt_sl[:].rearrange("(k c) x -> c k x", k=YPC),
                        q_sb[:].rearrange("c (k x) -> c k x", k=YPC),
                    )
                    qt_full = dpool.tile(
                        [H * C, W], F32, tag="qtfull", bufs=2,
                        addr_space="Shared", name=f"qtfull_{_rep}_{it}",
                    )
                    nc.gpsimd.collective_compute(
                        "AllGather",
                        mybir.AluOpType.bypass,
                        replica_groups=[list(range(NCORES))],
                        ins=[qt_sl[:]],
                        outs=[qt_full[:]],
                    )
                else:
                    nc.sync.dma_start(qt_out[:], q_sb[:])

    nc.compile()
    return nc


def _host_inputs(unaries, rgb, spatial_kernel, bilateral_kernel, compatibility_matrix):
    u = np.transpose(np.asarray(unaries, dtype=np.float32)[0], (2, 0, 1)).reshape(C, N)
    rgbf = np.asarray(rgb, dtype=np.float32)[0].reshape(N, 3)

    yy, xx = np.meshgrid(
        np.arange(H, dtype=np.float64), np.arange(W, dtype=np.float64), indexing="ij"
    )
    pos = np.stack([xx.ravel(), yy.ravel()], axis=1)  # [N, 2] (x, y)

    fb = np.concatenate(
        [pos / THETA_ALPHA, rgbf.astype(np.float64) / THETA_BETA], axis=1
    )
    fb -= fb.mean(axis=0, keepdims=True)  # centering: reduces fp32 cancellation
    fb32 = fb.astype(np.float32)
    sq = (fb32.astype(np.float64) ** 2).sum(axis=1)
    mhalf_sq = (-0.5 * sq).astype(np.float32)

    ubT = np.empty((7, N), np.float32)
    ubT[0:5] = fb32.T
    ubT[5] = mhalf_sq
    ubT[6] = 1.0
    vbT = np.empty((7, N), np.float32)
    vbT[0:5] = fb32.T
    vbT[5] = 1.0
    vbT[6] = mhalf_sq

    d = np.arange(-(H - 1), H, dtype=np.float64)
    g1tab = np.exp(-(d * d) / (2.0 * THETA_GAMMA**2))

    def g1(dd):
        return g1tab[np.asarray(dd) + (H - 1)]

    gx = g1(np.arange(W)[:, None] - np.arange(W)[None, :])  # [x, x']
    g2d_np = gx.astype(np.float32)
    s1 = np.array([g1(np.arange(H) - t).sum() for t in range(H)])  # exact ns factors

    comp = np.asarray(compatibility_matrix, dtype=np.float64)
    A_s = -(comp @ np.asarray(spatial_kernel, dtype=np.float64))
    A_b = -(comp @ np.asarray(bilateral_kernel, dtype=np.float64))
    awT_np = np.zeros((54, C), np.float32)
    awT_np[0:21] = A_s.T.astype(np.float32)
    awT_np[32:53] = A_b.T.astype(np.float32)

    qt0_np = np.ascontiguousarray(
        u.reshape(C, H, W).transpose(2, 1, 0).reshape(W, H * C)
    )

    in_maps = []
    for c in range(NCORES):
        sl = slice(c * COLS, (c + 1) * COLS)
        dy = np.arange(H)[:, None] - (YPC * c + np.arange(YPC))[None, :]  # [y, k]
        gm = g1(dy).astype(np.float32)  # [112, 14]
        gmat_np = np.ascontiguousarray(
            np.broadcast_to(gm[None], (W, H, YPC))
        ).reshape(W, H * YPC)
        v = 1.0 / (s1[YPC * c + np.arange(YPC)][None, :] * s1[np.arange(W)][:, None])
        invns_np = np.ascontiguousarray(
            np.repeat(v[:, :, None], C, axis=2).astype(np.float32)
        ).reshape(W, YPC * C)
        in_maps.append(
            dict(
                ubT=ubT,
                vbT_sl=np.ascontiguousarray(vbT[:, sl]),
                g2d=g2d_np,
                gmat=gmat_np,
                invns=invns_np,
                u_sl=np.ascontiguousarray(u[:, sl]),
                qt0=qt0_np,
                awT=awT_np,
            )
        )
    return in_maps


def run(inputs, trace=False, reps=1, **spmd_kwargs):
    in_maps = _host_inputs(**inputs)
    key = ("nc", reps)
    if key not in _CACHE:
        _CACHE[key] = _build_program(reps)
    nc = _CACHE[key]
    res = run_bass_kernel_spmd(
        nc, in_maps, core_ids=list(range(NCORES)), trace=trace, **spmd_kwargs
    )
    qs = [np.asarray(res.results[c]["qt_out"]) for c in range(NCORES)]
    Q = np.concatenate(qs, axis=1)  # [C, N]
    out = Q.reshape(C, H, W).transpose(1, 2, 0)[None].astype(np.float32)
    return out, res


def kernel(unaries, rgb, spatial_kernel, bilateral_kernel, compatibility_matrix):
    out, _ = run(
        dict(
            unaries=unaries,
            rgb=rgb,
            spatial_kernel=spatial_kernel,
            bilateral_kernel=bilateral_kernel,
            compatibility_matrix=compatibility_matrix,
        )
    )
    return out


# revision 10
# speedup vs baseline: 1.7700x; 1.2324x over previous
"""CRF-RNN layer (nn_CrfRnnLayer) as a Bass/Tile SPMD kernel on 8 TRN2 NeuronCores.

Algorithm (matches reference.py):
  N = 112*112 pixels, C = 21 classes, 5 mean-field iterations:
    sm = softmax(Q, axis=classes)
    spatial_out  = (sm @ Ks) / ns      Ks[i,j] = exp(-||p_i-p_j||^2 / (2*3^2))
    bilateral_out= (sm @ Kb) / nb      Kb from (pos/160, rgb/3) features
    Q = u - comp @ (sk @ spatial_out + bk @ bilateral_out)

Sharding: pixel columns sharded 8 ways (each core owns 14 image rows = 1568
pixels). Each core computes its column slice of both filtered outputs from the
fully replicated softmax; a per-iteration AllGather (131KB/rank) replicates Q.

Structure per core (instruction-count optimized):
  - Bilateral slice E_b [N, 1568] precomputed once on-device in bf16 (K=7
    matmul of augmented features -> -0.5*d2 into a 4-bank PSUM tile, one ACT
    exp over all 4 banks) and streamed back each iteration in 7-block batches.
  - Main bilateral matmul: 98 contraction blocks x 4 col-tiles into a single
    4-bank [33, 2048] PSUM accumulator; softmax lhsT carries a ones column at
    partition 32 whose PSUM row is the nb normalizer (read once, iteration 0).
  - Spatial kernel never materialized: Ks = Gy (x) Gx Kronecker product. One
    big DVE multiply (per half) scales the softmax image by the per-core
    y-Gaussian weights; 112 PE matmuls against the shared [112,112] x-Gaussian
    do the rest. Normalizers ns are exact separable sums computed on host.
  - Layout changes (pixel-major <-> block-major <-> class-major) via single
    strided DMAs through DRAM instead of per-tile PE transposes.
"""

import numpy as np

import concourse.mybir as mybir
import concourse.tile as tile
from concourse import bacc
from concourse.bass import _add_dep_helper
from concourse.bass_utils import run_bass_kernel_spmd

H = 112
W = 112
C = 21
N = H * W
NCORES = 8
YPC = H // NCORES            # 14 image rows per core
COLS = N // NCORES           # 1568 pixels per core
NB = 98                      # contraction blocks of 128 pixels
BB = 7                       # blocks per DMA batch
NBATCH = NB // BB            # 14
CTS = [(0, 512), (512, 512), (1024, 512), (1536, 32)]
NITER = 5
THETA_ALPHA = 160.0
THETA_BETA = 3.0
THETA_GAMMA = 3.0
CP = 33                      # padded lhsT width; col 32 is the ones column

F32 = mybir.dt.float32
BF16 = mybir.dt.bfloat16
EXPF = mybir.ActivationFunctionType.Exp

_CACHE = {}


def _build_program(reps=1):
    nc = bacc.Bacc("TRN2", target_bir_lowering=False, debug=False, num_devices=NCORES)

    # Chain every PE matmul in emission order (ordering-only deps) so the
    # scheduler keeps same-weights matmuls adjacent -> legalization dedups
    # the LDWEIGHTS instruction for consecutive same-lhsT matmuls.
    _mm_state = {"prev": None}

    def mm(*args, **kwargs):
        inst = nc.tensor.matmul(*args, **kwargs)
        if _mm_state["prev"] is not None:
            _add_dep_helper(inst.ins, _mm_state["prev"].ins, sync=False,
                            reason="pe emission order")
        _mm_state["prev"] = inst
        return inst

    ubT = nc.dram_tensor("ubT", [7, N], F32, kind="ExternalInput")
    vbT_sl = nc.dram_tensor("vbT_sl", [7, COLS], F32, kind="ExternalInput")
    g2d = nc.dram_tensor("g2d", [H, W], F32, kind="ExternalInput")
    gy2 = nc.dram_tensor("gy2", [H, YPC], F32, kind="ExternalInput")
    invns2 = nc.dram_tensor("invns2", [YPC, W * C], F32, kind="ExternalInput")
    u_sl = nc.dram_tensor("u_sl", [C, COLS], F32, kind="ExternalInput")
    qt0 = nc.dram_tensor("qt0", [W, H * C], F32, kind="ExternalInput")
    awT = nc.dram_tensor("awT", [54, C], F32, kind="ExternalInput")
    qt_out = nc.dram_tensor("qt_out", [C, COLS], F32, kind="ExternalOutput")

    with tile.TileContext(nc) as tc:
        with (
            tc.tile_pool(name="const", bufs=1) as cpool,
            tc.tile_pool(name="smx", bufs=1) as smpool,
            tc.tile_pool(name="stream", bufs=2) as stpool,
            tc.tile_pool(name="outp", bufs=1) as opool,
            tc.tile_pool(name="psum", bufs=1, space="PSUM") as pspool,
            tc.tile_pool(name="dram", bufs=1, space="DRAM") as dpool,
        ):
          for _rep in range(reps):
            # ---------------- constants ----------------
            vbT_sb = cpool.tile([7, COLS], F32, tag="vbT", name=f"vbT_{_rep}")
            nc.sync.dma_start(vbT_sb[:], vbT_sl[:])
            u_sb = cpool.tile([C, COLS], F32, tag="usb", name=f"usb_{_rep}")
            nc.sync.dma_start(u_sb[:], u_sl[:])
            awT_sb = cpool.tile([54, C], F32, tag="awT", name=f"awT_{_rep}")
            nc.sync.dma_start(awT_sb[:], awT[:])
            invns2_sb = cpool.tile([YPC, W * C], F32, tag="invns2", name=f"invns2_{_rep}")
            nc.sync.dma_start(invns2_sb[:], invns2[:])
            gy2_sb = cpool.tile([H, YPC], F32, tag="gy2", name=f"gy2_{_rep}")
            nc.sync.dma_start(gy2_sb[:], gy2[:])

            g2d_f = cpool.tile([H, W], F32, tag="g2df", name=f"g2df_{_rep}")
            nc.sync.dma_start(g2d_f[:], g2d[:])


            ones1 = cpool.tile([1, C], F32, tag="ones1", name=f"ones1_{_rep}")
            nc.gpsimd.memset(ones1[:], 1.0)
            invnb_bc = cpool.tile([C, COLS], F32, tag="invnb", name=f"invnb_{_rep}")

            # DRAM scratch
            e_b = dpool.tile([NB, 128, COLS], F32, tag="eb", name=f"eb_{_rep}")
            sm_d = dpool.tile([N, C], F32, tag="smd", name=f"smd_{_rep}")
            td_d = dpool.tile([H, W * C], F32, tag="td", name=f"td_{_rep}")
            spd = dpool.tile([W * C, YPC], F32, tag="spd", name=f"spd_{_rep}")

            # ---------------- precompute E_b (98 blocks, batches of 7) ------
            for bt in range(NBATCH):
                ub7 = stpool.tile([7, BB * 128], F32, tag="ub7", name=f"ub7_{_rep}_{bt}")
                nc.sync.dma_start(
                    ub7[:], ubT[:, bt * BB * 128 : (bt + 1) * BB * 128]
                )
                e7 = stpool.tile(
                    [128, BB, COLS], F32, tag="e7", bufs=1, name=f"e7p_{_rep}_{bt}"
                )
                for b in range(BB):
                    tagp = "blk" if b % 2 == 0 else "spq"
                    d2_ps = pspool.tile(
                        [128, 2048], F32, tag=tagp, name=f"d2_{_rep}_{bt}_{b}"
                    )
                    for ci, (c0, cw) in enumerate(CTS):
                        mm(
                            d2_ps[:, ci * 512 : ci * 512 + cw],
                            ub7[:, b * 128 : (b + 1) * 128],
                            vbT_sb[:, c0 : c0 + cw],
                            start=True,
                            stop=True,
                        )
                    nc.scalar.activation(e7[:, b, :], d2_ps[:, 0:COLS], EXPF)
                nc.sync.dma_start(
                    e_b[bt * BB : (bt + 1) * BB].rearrange("b r f -> r b f"), e7[:]
                )

            # ---------------- iterations ----------------
            qt_full = None
            for it in range(NITER):
                # ---- Q in [x, (y c)] layout
                q3f = smpool.tile([W, H * C], F32, tag="q3f", name=f"q3f_{_rep}_{it}")
                if it == 0:
                    nc.sync.dma_start(q3f[:], qt0[:])
                else:
                    nc.sync.dma_start(
                        q3f[:], qt_full[:].rearrange("r x -> x r")
                    )

                # ---- softmax over classes (no max subtraction; |Q| small)
                eq3b = smpool.tile([W, H * C], F32, tag="eq3b", name=f"eq_{_rep}_{it}")
                nc.scalar.activation(eq3b[:], q3f[:], EXPF)
                sums = smpool.tile([W, H], F32, tag="sums", name=f"sums_{_rep}_{it}")
                nc.vector.reduce_sum(
                    sums[:],
                    eq3b[:].rearrange("p (y c) -> p y c", y=H),
                    axis=mybir.AxisListType.X,
                )
                rsum = smpool.tile([W, H], F32, tag="rsum", name=f"rsum_{_rep}_{it}")
                nc.vector.reciprocal(rsum[:], sums[:])
                smT3 = smpool.tile([W, H, C], F32, tag="smT3", name=f"smT3_{_rep}_{it}")
                nc.vector.tensor_mul(
                    smT3[:],
                    eq3b[:].rearrange("p (y c) -> p y c", y=H),
                    rsum[:].broadcast_to([W, H, C]),
                )

                # ---- block-major softmax copy for the bilateral lhsT
                nc.sync.dma_start(
                    sm_d[:].rearrange("(y x) c -> x y c", x=W), smT3[:]
                )
                smB = smpool.tile([128, NB, CP], F32, tag="smB", name=f"smB_{_rep}_{it}")
                nc.gpsimd.memset(smB[:, :, C:CP], 1.0)
                nc.sync.dma_start(
                    smB[:, :, 0:C], sm_d[:].rearrange("(b r) c -> r b c", r=128)
                )

                # ---- spatial filtering: separable x-pass then y-pass
                # pass 1 (x-conv): T[x', (y c)] = Gx^T @ smT3
                p1a = pspool.tile([W, 2048], F32, tag="spq", name=f"p1a_{_rep}_{it}")
                smflat = smT3[:].rearrange("p y c -> p (y c)")
                for ci in range(4):
                    mm(p1a[:, ci * 512 : (ci + 1) * 512], g2d_f[:],
                       smflat[:, ci * 512 : (ci + 1) * 512], start=True, stop=True)
                p1b = pspool.tile([W, 304], F32, tag="spq", name=f"p1b_{_rep}_{it}")
                mm(p1b[:], g2d_f[:], smflat[:, 2048:2352], start=True, stop=True)
                t_sb = smpool.tile([W, H * C], F32, tag="tsb", name=f"tsb_{_rep}_{it}")
                nc.vector.tensor_copy(t_sb[:, 0:2048], p1a[:])
                nc.vector.tensor_copy(t_sb[:, 2048:2352], p1b[:])
                # bounce [x', (y c)] -> [y, (x' c)]
                nc.sync.dma_start(
                    td_d[:].rearrange("y (x c) -> x y c", x=W),
                    t_sb[:].rearrange("p (y c) -> p y c", y=H),
                )
                t2_sb = smpool.tile([H, W * C], F32, tag="t2sb", name=f"t2sb_{_rep}_{it}")
                nc.sync.dma_start(t2_sb[:], td_d[:])
                # pass 2 (y-conv): sp2[k, (x' c)] = gy2^T @ T2, then / ns
                p2a = pspool.tile([YPC, 2048], F32, tag="spq", name=f"p2a_{_rep}_{it}")
                for ci in range(4):
                    mm(p2a[:, ci * 512 : (ci + 1) * 512], gy2_sb[:],
                       t2_sb[:, ci * 512 : (ci + 1) * 512], start=True, stop=True)
                p2b = pspool.tile([YPC, 304], F32, tag="spq", name=f"p2b_{_rep}_{it}")
                mm(p2b[:], gy2_sb[:], t2_sb[:, 2048:2352], start=True, stop=True)
                sp2 = opool.tile([YPC, W * C], F32, tag="sp2", name=f"sp2_{_rep}_{it}")
                nc.vector.tensor_mul(sp2[:, 0:2048], p2a[:], invns2_sb[:, 0:2048])
                nc.vector.tensor_mul(sp2[:, 2048:2352], p2b[:], invns2_sb[:, 2048:2352])

                # ---- bilateral: stream E_b and accumulate [CP, 2048] PSUM
                bl_ps = pspool.tile([CP, 2048], F32, tag="blk", name=f"bl_{_rep}_{it}")
                for bt in range(NBATCH):
                    e7s = stpool.tile(
                        [128, BB, COLS], F32, tag="e7", bufs=1, name=f"e7s_{_rep}_{it}_{bt}"
                    )
                    nc.sync.dma_start(
                        e7s[:], e_b[bt * BB : (bt + 1) * BB].rearrange("b r f -> r b f")
                    )
                    for b in range(BB):
                        jb = bt * BB + b
                        for ci, (c0, cw) in enumerate(CTS):
                            mm(
                                bl_ps[:, ci * 512 : ci * 512 + cw],
                                smB[:, jb, :],
                                e7s[:, b, c0 : c0 + cw],
                                start=(jb == 0),
                                stop=(jb == NB - 1),
                            )

                # ---- iteration 0: build 1/nb broadcast across class partitions
                if it == 0:
                    nbrow = opool.tile([1, COLS], F32, tag="nbrow", name=f"nbrow_{_rep}")
                    nc.vector.tensor_copy(nbrow[:], bl_ps[32:33, 0:COLS])
                    rnb = opool.tile([1, COLS], F32, tag="rnb", name=f"rnb_{_rep}")
                    nc.vector.reciprocal(rnb[:], nbrow[:])
                    bc_ps = pspool.tile([C, 2048], F32, tag="spq", name=f"bc_{_rep}")
                    for ci, (c0, cw) in enumerate(CTS):
                        mm(
                            bc_ps[:, ci * 512 : ci * 512 + cw],
                            ones1[:],
                            rnb[0:1, c0 : c0 + cw],
                            start=True,
                            stop=True,
                        )
                    nc.vector.tensor_copy(invnb_bc[:], bc_ps[:, 0:COLS])

                # ---- stacked [54, COLS]: spatial_out rows 0:21, bilateral 32:53
                so54 = opool.tile([54, COLS], F32, tag="so54", name=f"so54_{_rep}_{it}")
                nc.gpsimd.memset(so54[:], 0.0)
                # 2D-transpose write: spd[(x c), k] <- sp2[k, (x c)]
                nc.sync.dma_start(spd[:].rearrange("r k -> k r"), sp2[:])
                # 3D read: X1[c, x, k] <- spd[(x c), k]
                x1 = opool.tile([C, W, YPC], F32, tag="x1", name=f"x1_{_rep}_{it}")
                nc.sync.dma_start(
                    x1[:], spd[:].rearrange("(x c) k -> c x k", x=W)
                )
                # DVE free-dim transpose (x,k) -> (k,x) into the stacked tile
                nc.vector.tensor_copy(
                    so54[0:C, :].rearrange("c (k x) -> c k x", k=YPC),
                    x1[:].rearrange("c x k -> c k x"),
                )
                # normalized bilateral into rows 32:53
                nc.vector.tensor_mul(so54[32:53, :], bl_ps[0:C, 0:COLS], invnb_bc[:])

                # ---- Q = u + [A_s ; A_b] @ [sp_out ; bl_out]
                q_ps = pspool.tile([C, 2048], F32, tag="spq", name=f"qps_{_rep}_{it}")
                for ci, (c0, cw) in enumerate(CTS):
                    mm(
                        q_ps[:, ci * 512 : ci * 512 + cw],
                        awT_sb[:],
                        so54[:, c0 : c0 + cw],
                        start=True,
                        stop=True,
                    )
                q_sb = opool.tile([C, COLS], F32, tag="qsb", name=f"qsb_{_rep}_{it}")
                nc.vector.tensor_add(q_sb[:], q_ps[:, 0:COLS], u_sb[:])

                # ---- publish Q: AllGather (iters 0-3) or final output
                if it < NITER - 1:
                    qt_sl = dpool.tile(
                        [YPC * C, W], F32, tag="qtsl", bufs=2, name=f"qtsl_{_rep}_{it}"
                    )
                    nc.sync.dma_start(
                        qt_sl[:].rearrange("(k c) x -> c k x", k=YPC),
                        q_sb[:].rearrange("c (k x) -> c k x", k=YPC),
                    )
                    qt_full = dpool.tile(
                        [H * C, W], F32, tag="qtfull", bufs=2,
                        addr_space="Shared", name=f"qtfull_{_rep}_{it}",
                    )
                    nc.gpsimd.collective_compute(
                        "AllGather",
                        mybir.AluOpType.bypass,
                        replica_groups=[list(range(NCORES))],
                        ins=[qt_sl[:]],
                        outs=[qt_full[:]],
                    )
                else:
                    nc.sync.dma_start(qt_out[:], q_sb[:])

    nc.compile()
    return nc


def _host_inputs(unaries, rgb, spatial_kernel, bilateral_kernel, compatibility_matrix):
    u = np.transpose(np.asarray(unaries, dtype=np.float32)[0], (2, 0, 1)).reshape(C, N)
    rgbf = np.asarray(rgb, dtype=np.float32)[0].reshape(N, 3)

    yy, xx = np.meshgrid(
        np.arange(H, dtype=np.float64), np.arange(W, dtype=np.float64), indexing="ij"
    )
    pos = np.stack([xx.ravel(), yy.ravel()], axis=1)  # [N, 2] (x, y)

    fb = np.concatenate(
        [pos / THETA_ALPHA, rgbf.astype(np.float64) / THETA_BETA], axis=1
    )
    fb -= fb.mean(axis=0, keepdims=True)  # centering: reduces fp32 cancellation
    fb32 = fb.astype(np.float32)
    sq = (fb32.astype(np.float64) ** 2).sum(axis=1)
    mhalf_sq = (-0.5 * sq).astype(np.float32)

    ubT = np.empty((7, N), np.float32)
    ubT[0:5] = fb32.T
    ubT[5] = mhalf_sq
    ubT[6] = 1.0
    vbT = np.empty((7, N), np.float32)
    vbT[0:5] = fb32.T
    vbT[5] = 1.0
    vbT[6] = mhalf_sq

    d = np.arange(-(H - 1), H, dtype=np.float64)
    g1tab = np.exp(-(d * d) / (2.0 * THETA_GAMMA**2))

    def g1(dd):
        return g1tab[np.asarray(dd) + (H - 1)]

    gx = g1(np.arange(W)[:, None] - np.arange(W)[None, :])  # [x, x']
    g2d_np = gx.astype(np.float32)
    s1 = np.array([g1(np.arange(H) - t).sum() for t in range(H)])  # exact ns factors

    comp = np.asarray(compatibility_matrix, dtype=np.float64)
    A_s = -(comp @ np.asarray(spatial_kernel, dtype=np.float64))
    A_b = -(comp @ np.asarray(bilateral_kernel, dtype=np.float64))
    awT_np = np.zeros((54, C), np.float32)
    awT_np[0:21] = A_s.T.astype(np.float32)
    awT_np[32:53] = A_b.T.astype(np.float32)

    qt0_np = np.ascontiguousarray(
        u.reshape(C, H, W).transpose(2, 1, 0).reshape(W, H * C)
    )

    in_maps = []
    for c in range(NCORES):
        sl = slice(c * COLS, (c + 1) * COLS)
        dy = np.arange(H)[:, None] - (YPC * c + np.arange(YPC))[None, :]  # [y, k]
        gy2_np = np.ascontiguousarray(g1(dy).astype(np.float32))  # [112, 14]
        # invns2[k, x*21 + cc] = 1 / (s1[y0+k] * s1[x])
        v = 1.0 / (s1[YPC * c + np.arange(YPC)][:, None] * s1[np.arange(W)][None, :])
        invns2_np = np.ascontiguousarray(
            np.repeat(v[:, :, None], C, axis=2).astype(np.float32)
        ).reshape(YPC, W * C)
        in_maps.append(
            dict(
                ubT=ubT,
                vbT_sl=np.ascontiguousarray(vbT[:, sl]),
                g2d=g2d_np,
                gy2=gy2_np,
                invns2=invns2_np,
                u_sl=np.ascontiguousarray(u[:, sl]),
                qt0=qt0_np,
                awT=awT_np,
            )
        )
    return in_maps


def run(inputs, trace=False, reps=1, **spmd_kwargs):
    in_maps = _host_inputs(**inputs)
    key = ("nc", reps)
    if key not in _CACHE:
        _CACHE[key] = _build_program(reps)
    nc = _CACHE[key]
    res = run_bass_kernel_spmd(
        nc, in_maps, core_ids=list(range(NCORES)), trace=trace, **spmd_kwargs
    )
    qs = [np.asarray(res.results[c]["qt_out"]) for c in range(NCORES)]
    Q = np.concatenate(qs, axis=1)  # [C, N]
    out = Q.reshape(C, H, W).transpose(1, 2, 0)[None].astype(np.float32)
    return out, res


def kernel(unaries, rgb, spatial_kernel, bilateral_kernel, compatibility_matrix):
    out, _ = run(
        dict(
            unaries=unaries,
            rgb=rgb,
            spatial_kernel=spatial_kernel,
            bilateral_kernel=bilateral_kernel,
            compatibility_matrix=compatibility_matrix,
        )
    )
    return out
